# revision 25
# baseline (speedup 1.0000x reference)
"""Bahdanau additive attention on 8 TRN2 NeuronCores.

Sharding: pure data-parallel over batch B=32 -> 4 batches/core, no collectives.

Per-core kernel (BS=4, S=2048, H=1024), per batch b, per s-tile t (512):
  PE:  v_projT[ho] = sum_k Wv[k,ho]^T @ valuesT[k, t]   (f32r, N=512)
  ACT: featT = tanh(v_projT + (q_proj[b] + bv) per-partition bias)  -> f32r
  PE:  scores[1,512] += Wc[ho]^T @ featT
  DVE: scores += mask_bias;  ACT: p_t = exp(scores_t), accum_out -> tile sum
  PE:  broadcast p_t to 128 partitions (ones[1,128]^T @ p_t outer product)
  DVE: ctx_acc[:, k,t] = sum_j valuesT[k-slice] * p_t   (tensor_tensor_reduce)
Tail per batch: Z = sum(tile sums); w = p/Z (ACT scale); unnormalized ctxT and
Z are exported, the final ctx/Z division happens host-side during unsharding.
No max-subtraction is needed: |scores| <= ||Wc||_1 + eps ~ 16, exp is safe in f32,
which makes softmax+context tile-local (single pass over values, no re-read).
bc is dropped: softmax and context are invariant to a scalar score offset.
"""

import sys

for _p in ("/opt/trn_rl_repo",):
    if _p not in sys.path:
        sys.path.insert(0, _p)

import numpy as np

import concourse.bass as bass  # noqa: F401
import concourse.mybir as mybir
import concourse.tile as tile
from concourse import bacc
from concourse.bass_utils import run_bass_kernel_spmd

B, S, H = 32, 2048, 1024
NCORES = 8
BS = B // NCORES  # batches per core
KC = H // 128     # 8 contraction chunks
HO = H // 128     # 8 output-h chunks
ST = S // 512     # 4 s-tiles
F32 = mybir.dt.float32
F32R = mybir.dt.float32r
BF16 = mybir.dt.bfloat16

TRACE = False
LAST_RESULT = None

_cache = {}


def _build():
    AF = mybir.ActivationFunctionType
    ALU = mybir.AluOpType
    AX = mybir.AxisListType

    nc = bacc.Bacc("TRN2", target_bir_lowering=False, debug=False,
                   num_devices=NCORES)

    valT = nc.declare_dram_parameter("valT", [BS, H, S], BF16, isOutput=False).ap()
    qt = nc.declare_dram_parameter("qt", [128, KC * BS], BF16, isOutput=False).ap()
    wv = nc.declare_dram_parameter("wv", [128, KC * H], BF16, isOutput=False).ap()
    wq = nc.declare_dram_parameter("wq", [128, KC * H], BF16, isOutput=False).ap()
    wc = nc.declare_dram_parameter("wc", [128, KC], BF16, isOutput=False).ap()
    bqv = nc.declare_dram_parameter("bqv", [128, KC], F32, isOutput=False).ap()
    mkb = nc.declare_dram_parameter("mkb", [BS, S], F32, isOutput=False).ap()
    octx = nc.declare_dram_parameter("octx", [BS, 128, KC], F32, isOutput=True).ap()
    owgt = nc.declare_dram_parameter("owgt", [BS, S], F32, isOutput=True).ap()
    oz = nc.declare_dram_parameter("oz", [BS, 1], F32, isOutput=True).ap()

    with tile.TileContext(nc) as tc:
        import contextlib

        with contextlib.ExitStack() as es:
            const = es.enter_context(tc.tile_pool(name="const", bufs=1))
            psv = es.enter_context(tc.tile_pool(name="psv", bufs=5, space="PSUM"))
            pss = es.enter_context(tc.tile_pool(name="pss", bufs=1, space="PSUM"))
            psp = es.enter_context(tc.tile_pool(name="psp", bufs=2, space="PSUM"))

            # scalar queue: tiny consts first, then wv chunks, then vt stream.
            wc_sb = const.tile([128, KC], BF16, name="wc_sb")
            nc.scalar.dma_start(wc_sb[:], wc[:])
            bqv_sb = const.tile([128, KC], F32, name="bqv_sb")
            nc.scalar.dma_start(bqv_sb[:], bqv[:])
            qt_sb = const.tile([128, KC * BS], BF16, name="qt_sb")
            nc.scalar.dma_start(qt_sb[:], qt[:])
            ones_dram = nc.inline_tensor(np.ones((1, 128), np.float32), name="ones_c")
            ones_sb = const.tile([1, 128], F32R, name="ones_sb")
            nc.scalar.dma_start(ones_sb[:], ones_dram.ap().bitcast(F32R))
            qp_sb = const.tile([128, HO * BS], F32, name="qp_sb")
            valt_pool = es.enter_context(tc.tile_pool(name="valt", bufs=3))
            vt00 = valt_pool.tile([128, KC * 512], BF16, name="vt", tag="vt")
            nc.scalar.dma_start(
                vt00.rearrange("p (k j) -> p k j", k=KC),
                valT[0:1, :, 0:512].rearrange("o (k p) j -> (o p) k j", p=128),
            )
            wv_sb = const.tile([128, KC * H], BF16, name="wv_sb")
            for ho in range(HO):
                nc.scalar.dma_start(wv_sb[:, ho * H:(ho + 1) * H],
                                    wv[:, ho * H:(ho + 1) * H])

            # ---- q_projT: q_proj = query @ Wq, transpose via DRAM round-trip ----
            dramp = es.enter_context(tc.tile_pool(name="dramp", bufs=1, space="DRAM"))
            qpd = dramp.tile([BS, H], F32, name="qpd")
            with tc.tile_pool(name="wqp", bufs=1) as wqp:
                wq_sb = wqp.tile([128, KC * H], BF16, name="wq_sb")
                nc.sync.dma_start(wq_sb[:], wq[:])
                qpn = wqp.tile([BS, H], F32, name="qpn")
                for n in range(2):
                    ps_q = psv.tile([BS, 512], F32, name="ps_q", tag="psv")
                    for k in range(KC):
                        nc.tensor.matmul(
                            ps_q[:],
                            qt_sb[:, k * BS:(k + 1) * BS],
                            wq_sb[:, k * H + n * 512: k * H + n * 512 + 512],
                            start=(k == 0), stop=(k == KC - 1),
                        )
                    nc.vector.tensor_copy(qpn[:, n * 512:(n + 1) * 512], ps_q[:])
                nc.sync.dma_start(qpd[:], qpn[:])
            qpr = const.tile([128, HO * BS], F32, name="qpr")
            for c in range(HO):
                nc.sync.dma_start(
                    qpr[:, c * BS:(c + 1) * BS],
                    qpd[:, c * 128:(c + 1) * 128].rearrange("b p -> p b"),
                )
            for c in range(HO):
                nc.vector.tensor_scalar_add(
                    qp_sb[:, c * BS:(c + 1) * BS],
                    qpr[:, c * BS:(c + 1) * BS],
                    bqv_sb[:, c:c + 1],
                )

            ft_pool = es.enter_context(tc.tile_pool(name="ftp", bufs=10))
            sm_pool = es.enter_context(tc.tile_pool(name="sm", bufs=2))
            cx_pool = es.enter_context(tc.tile_pool(name="cx", bufs=2))

            for b in range(BS):
                mb_sb = sm_pool.tile([1, S], F32, name="mb_sb", tag="mb")
                nc.sync.dma_start(mb_sb[:], mkb[b:b + 1, :])
                p_sb = sm_pool.tile([1, S], F32R, name="p_sb", tag="p")
                sums = sm_pool.tile([1, ST], F32, name="sums", tag="sums")
                ctr = cx_pool.tile([128, KC], F32, name="ctr", tag="ctr")

                for t in range(ST):
                    if b == 0 and t == 0:
                        vt = vt00
                    else:
                        vt = valt_pool.tile([128, KC * 512], BF16, name="vt",
                                            tag="vt")
                        nc.scalar.dma_start(
                            vt.rearrange("p (k j) -> p k j", k=KC),
                            valT[b:b + 1, :, t * 512:(t + 1) * 512]
                            .rearrange("o (k p) j -> (o p) k j", p=128),
                        )
                    ps_s = pss.tile([1, 512], F32, name="ps_s", tag="pss")
                    fts = []
                    for ho in range(HO):
                        ps_v = psv.tile([128, 512], F32, name="ps_v", tag="psv")
                        for k in range(KC):
                            nc.tensor.matmul(
                                ps_v[:],
                                wv_sb[:, ho * H + k * 128: ho * H + k * 128 + 128],
                                vt[:, k * 512:(k + 1) * 512],
                                start=(k == 0), stop=(k == KC - 1),
                            )
                        ft = ft_pool.tile([128, 512], BF16, name="ft", tag="ft")
                        nc.scalar.activation(
                            ft[:], ps_v[:], AF.Tanh,
                            bias=qp_sb[:, ho * BS + b: ho * BS + b + 1],
                        )
                        fts.append(ft)
                    for ho in range(HO):
                        nc.tensor.matmul(
                            ps_s[:], wc_sb[:, ho:ho + 1], fts[ho][:],
                            start=(ho == 0), stop=(ho == HO - 1),
                        )
                    # scores -> masked -> exp (with per-tile sum) -> broadcast
                    sc_t = sm_pool.tile([1, 512], F32, name="sc_t", tag="sct")
                    nc.vector.tensor_add(
                        sc_t[:], ps_s[:], mb_sb[:, t * 512:(t + 1) * 512])
                    nc.scalar.activation(
                        p_sb[:, t * 512:(t + 1) * 512], sc_t[:], AF.Exp,
                        accum_out=sums[:, t:t + 1],
                    )
                    prep = psp.tile([128, 512], F32, name="prep", tag="prep")
                    nc.tensor.matmul(
                        prep[:], ones_sb[:], p_sb[:, t * 512:(t + 1) * 512],
                        start=True, stop=True)
                    tmp = cx_pool.tile([128, KC * 512], F32, name="tmp", tag="tmp")
                    ctat = cx_pool.tile([128, KC], F32, name="ctat", tag="ctat")
                    kk = KC // 2
                    for h2 in range(2):
                        sl = slice(h2 * kk * 512, (h2 + 1) * kk * 512)
                        nc.vector.tensor_mul(
                            tmp[:, sl].rearrange("p (k j) -> p k j", k=kk),
                            vt[:, sl].rearrange("p (k j) -> p k j", k=kk),
                            prep.rearrange("p (o j) -> p o j", o=1)
                            .broadcast_to([128, kk, 512]),
                        )
                        nc.vector.tensor_reduce(
                            ctat[:, h2 * kk:(h2 + 1) * kk],
                            tmp[:, sl].rearrange("p (k j) -> p k j", k=kk),
                            axis=AX.X, op=ALU.add,
                        )
                    if t == 0:
                        nc.vector.tensor_copy(ctr[:], ctat[:])
                    else:
                        nc.vector.tensor_add(ctr[:], ctr[:], ctat[:])

                # ---------- batch tail: normalize ----------
                tot = sm_pool.tile([1, 1], F32, name="tot", tag="tot")
                nc.vector.tensor_reduce(tot[:], sums[:], axis=AX.X, op=ALU.add)
                recip = sm_pool.tile([1, 1], F32, name="recip", tag="recip")
                nc.vector.reciprocal(recip[:], tot[:])
                w_sb = sm_pool.tile([1, S], F32, name="w_sb", tag="w")
                nc.scalar.mul(w_sb[:], p_sb.bitcast(F32)[:], recip[:, 0:1])
                nc.sync.dma_start(owgt[b:b + 1, :], w_sb[:])
                nc.sync.dma_start(oz[b:b + 1, :], tot[:])

                nc.sync.dma_start(
                    octx[b:b + 1].rearrange("o p k -> (o p) k"),
                    ctr[:],
                )

    nc.compile()
    return nc


def _prep_host(query, values, mask, Wq, bq, Wv, bv, Wc, bc):
    f32 = np.float32
    query = np.asarray(query, f32)
    values = np.asarray(values, f32)
    mask = np.asarray(mask)
    Wq = np.asarray(Wq, f32)
    Wv = np.asarray(Wv, f32)
    Wc = np.asarray(Wc, f32)
    bq = np.asarray(bq, f32)
    bv = np.asarray(bv, f32)

    import ml_dtypes
    bf16 = ml_dtypes.bfloat16
    # wv: ho-major chunked layout [p, ho*H + k*128 + c] = Wv[k*128+p, ho*128+c]
    wv_l = np.ascontiguousarray(
        Wv.reshape(KC, 128, HO, 128).transpose(1, 2, 0, 3)
        .reshape(128, KC * H).astype(bf16))
    wq_l = np.ascontiguousarray(
        Wq.reshape(KC, 128, H).transpose(1, 0, 2).reshape(128, KC * H)
        .astype(bf16))
    wc_l = np.ascontiguousarray(Wc.reshape(KC, 128).T.astype(bf16))
    bqv_l = np.ascontiguousarray((bq + bv).reshape(KC, 128).T)  # [128, KC]

    valuesT = np.ascontiguousarray(values.transpose(0, 2, 1).astype(bf16))
    mask_bias = np.where(mask, 0.0, -1e38).astype(f32)          # [B, S]

    in_maps = []
    for c in range(NCORES):
        lo, hi = c * BS, (c + 1) * BS
        qs = query[lo:hi]
        qt_l = np.ascontiguousarray(
            qs.reshape(BS, KC, 128).transpose(2, 1, 0).reshape(128, KC * BS)
            .astype(bf16))
        in_maps.append({
            "valT": np.ascontiguousarray(valuesT[lo:hi]),
            "qt": qt_l,
            "wv": wv_l,
            "wq": wq_l,
            "wc": wc_l,
            "bqv": bqv_l,
            "mkb": np.ascontiguousarray(mask_bias[lo:hi]),
        })
    return in_maps


def kernel(query, values, mask, Wq, bq, Wv, bv, Wc, bc):
    global LAST_RESULT
    if "nc" not in _cache:
        _cache["nc"] = _build()
    nc = _cache["nc"]

    in_maps = _prep_host(query, values, mask, Wq, bq, Wv, bv, Wc, bc)
    res = run_bass_kernel_spmd(nc, in_maps, core_ids=list(range(NCORES)),
                               trace=TRACE)
    LAST_RESULT = res
    outs = res.results
    context = np.concatenate([outs[c]["octx"] for c in range(NCORES)], axis=0)
    context = context.transpose(0, 2, 1).reshape(B, H)  # [b,p,k] -> h = k*128+p
    weights = np.concatenate([outs[c]["owgt"] for c in range(NCORES)], axis=0)
    z = np.concatenate([outs[c]["oz"] for c in range(NCORES)], axis=0)
    context = context / z
    return context.astype(np.float32), weights.astype(np.float32)


# revision 26
# speedup vs baseline: 1.1036x; 1.1036x over previous
"""Bahdanau additive attention on 8 TRN2 NeuronCores.

Sharding: pure data-parallel over batch B=32 -> 4 batches/core, no collectives.

Per-core kernel (BS=4, S=2048, H=1024), per batch b, per s-tile t (512):
  PE:  v_projT[ho] = sum_k Wv[k,ho]^T @ valuesT[k, t]   (f32r, N=512)
  ACT: featT = tanh(v_projT + (q_proj[b] + bv) per-partition bias)  -> f32r
  PE:  scores[1,512] += Wc[ho]^T @ featT
  DVE: scores += mask_bias;  ACT: p_t = exp(scores_t), accum_out -> tile sum
  PE:  broadcast p_t to 128 partitions (ones[1,128]^T @ p_t outer product)
  DVE: ctx_acc[:, k,t] = sum_j valuesT[k-slice] * p_t   (tensor_tensor_reduce)
Tail per batch: Z = sum(tile sums); w = p/Z (ACT scale); unnormalized ctxT and
Z are exported, the final ctx/Z division happens host-side during unsharding.
No max-subtraction is needed: |scores| <= ||Wc||_1 + eps ~ 16, exp is safe in f32,
which makes softmax+context tile-local (single pass over values, no re-read).
bc is dropped: softmax and context are invariant to a scalar score offset.
"""

import sys

for _p in ("/opt/trn_rl_repo",):
    if _p not in sys.path:
        sys.path.insert(0, _p)

import numpy as np

import concourse.bass as bass  # noqa: F401
import concourse.mybir as mybir
import concourse.tile as tile
from concourse import bacc
from concourse.bass_utils import run_bass_kernel_spmd

B, S, H = 32, 2048, 1024
NCORES = 8
BS = B // NCORES  # batches per core
KC = H // 128     # 8 contraction chunks
HO = H // 128     # 8 output-h chunks
ST = S // 512     # 4 s-tiles
F32 = mybir.dt.float32
F32R = mybir.dt.float32r
BF16 = mybir.dt.bfloat16

TRACE = False
LAST_RESULT = None

_cache = {}


def _build():
    AF = mybir.ActivationFunctionType
    ALU = mybir.AluOpType
    AX = mybir.AxisListType

    nc = bacc.Bacc("TRN2", target_bir_lowering=False, debug=False,
                   num_devices=NCORES)

    valT = nc.declare_dram_parameter("valT", [BS, H, S], BF16, isOutput=False).ap()
    qt = nc.declare_dram_parameter("qt", [128, KC * BS], BF16, isOutput=False).ap()
    wv = nc.declare_dram_parameter("wv", [128, KC * H], BF16, isOutput=False).ap()
    wq = nc.declare_dram_parameter("wq", [128, KC * H], BF16, isOutput=False).ap()
    wc = nc.declare_dram_parameter("wc", [128, KC], BF16, isOutput=False).ap()
    bqv = nc.declare_dram_parameter("bqv", [128, KC], F32, isOutput=False).ap()
    mkb = nc.declare_dram_parameter("mkb", [BS, S], F32, isOutput=False).ap()
    octx = nc.declare_dram_parameter("octx", [BS, 128, KC], F32, isOutput=True).ap()
    owgt = nc.declare_dram_parameter("owgt", [BS, S], F32, isOutput=True).ap()
    oz = nc.declare_dram_parameter("oz", [BS, 1], F32, isOutput=True).ap()

    with tile.TileContext(nc) as tc:
        import contextlib

        with contextlib.ExitStack() as es:
            const = es.enter_context(tc.tile_pool(name="const", bufs=1))
            psv = es.enter_context(tc.tile_pool(name="psv", bufs=5, space="PSUM"))
            pss = es.enter_context(tc.tile_pool(name="pss", bufs=1, space="PSUM"))
            psp = es.enter_context(tc.tile_pool(name="psp", bufs=2, space="PSUM"))

            # scalar queue: tiny consts first, then wv chunks, then vt stream.
            wc_sb = const.tile([128, KC], BF16, name="wc_sb")
            nc.scalar.dma_start(wc_sb[:], wc[:])
            bqv_sb = const.tile([128, KC], F32, name="bqv_sb")
            nc.scalar.dma_start(bqv_sb[:], bqv[:])
            qt_sb = const.tile([128, KC * BS], BF16, name="qt_sb")
            nc.scalar.dma_start(qt_sb[:], qt[:])
            ones_dram = nc.inline_tensor(np.ones((1, 128), np.float32), name="ones_c")
            ones_sb = const.tile([1, 128], F32R, name="ones_sb")
            nc.scalar.dma_start(ones_sb[:], ones_dram.ap().bitcast(F32R))
            qp_sb = const.tile([128, HO * BS], F32, name="qp_sb")
            valt_pool = es.enter_context(tc.tile_pool(name="valt", bufs=3))
            vt00 = valt_pool.tile([128, KC * 512], BF16, name="vt", tag="vt")
            nc.scalar.dma_start(
                vt00.rearrange("p (k j) -> p k j", k=KC),
                valT[0:1, :, 0:512].rearrange("o (k p) j -> (o p) k j", p=128),
            )
            wv_sb = const.tile([128, KC * H], BF16, name="wv_sb")
            for ho in range(HO):
                nc.scalar.dma_start(wv_sb[:, ho * H:(ho + 1) * H],
                                    wv[:, ho * H:(ho + 1) * H])

            # ---- q_projT: q_proj = query @ Wq, transpose via DRAM round-trip ----
            dramp = es.enter_context(tc.tile_pool(name="dramp", bufs=1, space="DRAM"))
            qpd = dramp.tile([BS, H], F32, name="qpd")
            with tc.tile_pool(name="wqp", bufs=1) as wqp:
                wq_sb = wqp.tile([128, KC * H], BF16, name="wq_sb")
                nc.sync.dma_start(wq_sb[:], wq[:])
                qpn = wqp.tile([BS, H], F32, name="qpn")
                for n in range(2):
                    ps_q = psv.tile([BS, 512], F32, name="ps_q", tag="psv")
                    for k in range(KC):
                        nc.tensor.matmul(
                            ps_q[:],
                            qt_sb[:, k * BS:(k + 1) * BS],
                            wq_sb[:, k * H + n * 512: k * H + n * 512 + 512],
                            start=(k == 0), stop=(k == KC - 1),
                        )
                    nc.vector.tensor_copy(qpn[:, n * 512:(n + 1) * 512], ps_q[:])
                nc.sync.dma_start(qpd[:], qpn[:])
            qpr = const.tile([128, HO * BS], F32, name="qpr")
            for c in range(HO):
                nc.sync.dma_start(
                    qpr[:, c * BS:(c + 1) * BS],
                    qpd[:, c * 128:(c + 1) * 128].rearrange("b p -> p b"),
                )
            for c in range(HO):
                nc.vector.tensor_scalar_add(
                    qp_sb[:, c * BS:(c + 1) * BS],
                    qpr[:, c * BS:(c + 1) * BS],
                    bqv_sb[:, c:c + 1],
                )

            ft_pool = es.enter_context(tc.tile_pool(name="ftp", bufs=4))
            sm_pool = es.enter_context(tc.tile_pool(name="sm", bufs=2))
            cx_pool = es.enter_context(tc.tile_pool(name="cx", bufs=2))

            for b in range(BS):
                mb_sb = sm_pool.tile([1, S], F32, name="mb_sb", tag="mb")
                nc.sync.dma_start(mb_sb[:], mkb[b:b + 1, :])
                p_sb = sm_pool.tile([1, S], F32R, name="p_sb", tag="p")
                sums = sm_pool.tile([1, ST], F32, name="sums", tag="sums")
                ctr = cx_pool.tile([128, KC], F32, name="ctr", tag="ctr")

                for t in range(ST):
                    if b == 0 and t == 0:
                        vt = vt00
                    else:
                        vt = valt_pool.tile([128, KC * 512], BF16, name="vt",
                                            tag="vt")
                        nc.scalar.dma_start(
                            vt.rearrange("p (k j) -> p k j", k=KC),
                            valT[b:b + 1, :, t * 512:(t + 1) * 512]
                            .rearrange("o (k p) j -> (o p) k j", p=128),
                        )
                    ps_s = pss.tile([1, 512], F32, name="ps_s", tag="pss")
                    fts = []
                    for ho in range(HO):
                        ps_v = psv.tile([128, 512], F32, name="ps_v", tag="psv")
                        for k in range(KC):
                            nc.tensor.matmul(
                                ps_v[:],
                                wv_sb[:, ho * H + k * 128: ho * H + k * 128 + 128],
                                vt[:, k * 512:(k + 1) * 512],
                                start=(k == 0), stop=(k == KC - 1),
                            )
                        ft = ft_pool.tile([128, 512], BF16, name="ft", tag="ft")
                        nc.scalar.activation(
                            ft[:], ps_v[:], AF.Tanh,
                            bias=qp_sb[:, ho * BS + b: ho * BS + b + 1],
                        )
                        fts.append(ft)
                        # scores MM for ho-1: its tanh finished during this group
                        if ho >= 1:
                            nc.tensor.matmul(
                                ps_s[:], wc_sb[:, ho - 1:ho], fts[ho - 1][:],
                                start=(ho == 1), stop=False,
                            )
                    nc.tensor.matmul(
                        ps_s[:], wc_sb[:, HO - 1:HO], fts[HO - 1][:],
                        start=False, stop=True,
                    )
                    # scores -> masked -> exp (with per-tile sum) -> broadcast
                    sc_t = sm_pool.tile([1, 512], F32, name="sc_t", tag="sct")
                    nc.vector.tensor_add(
                        sc_t[:], ps_s[:], mb_sb[:, t * 512:(t + 1) * 512])
                    nc.scalar.activation(
                        p_sb[:, t * 512:(t + 1) * 512], sc_t[:], AF.Exp,
                        accum_out=sums[:, t:t + 1],
                    )
                    prep = psp.tile([128, 512], F32, name="prep", tag="prep")
                    nc.tensor.matmul(
                        prep[:], ones_sb[:], p_sb[:, t * 512:(t + 1) * 512],
                        start=True, stop=True)
                    tmp = cx_pool.tile([128, KC * 512], F32, name="tmp", tag="tmp")
                    ctat = cx_pool.tile([128, KC], F32, name="ctat", tag="ctat")
                    kk = KC // 2
                    for h2 in range(2):
                        sl = slice(h2 * kk * 512, (h2 + 1) * kk * 512)
                        nc.vector.tensor_mul(
                            tmp[:, sl].rearrange("p (k j) -> p k j", k=kk),
                            vt[:, sl].rearrange("p (k j) -> p k j", k=kk),
                            prep.rearrange("p (o j) -> p o j", o=1)
                            .broadcast_to([128, kk, 512]),
                        )
                        nc.vector.tensor_reduce(
                            ctat[:, h2 * kk:(h2 + 1) * kk],
                            tmp[:, sl].rearrange("p (k j) -> p k j", k=kk),
                            axis=AX.X, op=ALU.add,
                        )
                    if t == 0:
                        nc.vector.tensor_copy(ctr[:], ctat[:])
                    else:
                        nc.vector.tensor_add(ctr[:], ctr[:], ctat[:])

                # ---------- batch tail: normalize ----------
                tot = sm_pool.tile([1, 1], F32, name="tot", tag="tot")
                nc.vector.tensor_reduce(tot[:], sums[:], axis=AX.X, op=ALU.add)
                recip = sm_pool.tile([1, 1], F32, name="recip", tag="recip")
                nc.vector.reciprocal(recip[:], tot[:])
                w_sb = sm_pool.tile([1, S], F32, name="w_sb", tag="w")
                nc.scalar.mul(w_sb[:], p_sb.bitcast(F32)[:], recip[:, 0:1])
                nc.sync.dma_start(owgt[b:b + 1, :], w_sb[:])
                nc.sync.dma_start(oz[b:b + 1, :], tot[:])

                nc.sync.dma_start(
                    octx[b:b + 1].rearrange("o p k -> (o p) k"),
                    ctr[:],
                )

    nc.compile()
    return nc


def _prep_host(query, values, mask, Wq, bq, Wv, bv, Wc, bc):
    f32 = np.float32
    query = np.asarray(query, f32)
    values = np.asarray(values, f32)
    mask = np.asarray(mask)
    Wq = np.asarray(Wq, f32)
    Wv = np.asarray(Wv, f32)
    Wc = np.asarray(Wc, f32)
    bq = np.asarray(bq, f32)
    bv = np.asarray(bv, f32)

    import ml_dtypes
    bf16 = ml_dtypes.bfloat16
    # wv: ho-major chunked layout [p, ho*H + k*128 + c] = Wv[k*128+p, ho*128+c]
    wv_l = np.ascontiguousarray(
        Wv.reshape(KC, 128, HO, 128).transpose(1, 2, 0, 3)
        .reshape(128, KC * H).astype(bf16))
    wq_l = np.ascontiguousarray(
        Wq.reshape(KC, 128, H).transpose(1, 0, 2).reshape(128, KC * H)
        .astype(bf16))
    wc_l = np.ascontiguousarray(Wc.reshape(KC, 128).T.astype(bf16))
    bqv_l = np.ascontiguousarray((bq + bv).reshape(KC, 128).T)  # [128, KC]

    valuesT = np.ascontiguousarray(values.transpose(0, 2, 1).astype(bf16))
    mask_bias = np.where(mask, 0.0, -1e38).astype(f32)          # [B, S]

    in_maps = []
    for c in range(NCORES):
        lo, hi = c * BS, (c + 1) * BS
        qs = query[lo:hi]
        qt_l = np.ascontiguousarray(
            qs.reshape(BS, KC, 128).transpose(2, 1, 0).reshape(128, KC * BS)
            .astype(bf16))
        in_maps.append({
            "valT": np.ascontiguousarray(valuesT[lo:hi]),
            "qt": qt_l,
            "wv": wv_l,
            "wq": wq_l,
            "wc": wc_l,
            "bqv": bqv_l,
            "mkb": np.ascontiguousarray(mask_bias[lo:hi]),
        })
    return in_maps


def kernel(query, values, mask, Wq, bq, Wv, bv, Wc, bc):
    global LAST_RESULT
    if "nc" not in _cache:
        _cache["nc"] = _build()
    nc = _cache["nc"]

    in_maps = _prep_host(query, values, mask, Wq, bq, Wv, bv, Wc, bc)
    res = run_bass_kernel_spmd(nc, in_maps, core_ids=list(range(NCORES)),
                               trace=TRACE)
    LAST_RESULT = res
    outs = res.results
    context = np.concatenate([outs[c]["octx"] for c in range(NCORES)], axis=0)
    context = context.transpose(0, 2, 1).reshape(B, H)  # [b,p,k] -> h = k*128+p
    weights = np.concatenate([outs[c]["owgt"] for c in range(NCORES)], axis=0)
    z = np.concatenate([outs[c]["oz"] for c in range(NCORES)], axis=0)
    context = context / z
    return context.astype(np.float32), weights.astype(np.float32)


# revision 31
# speedup vs baseline: 1.1225x; 1.0172x over previous
"""Bahdanau additive attention on 8 TRN2 NeuronCores.

Sharding: pure data-parallel over batch B=32 -> 4 batches/core, no collectives.

Per-core kernel (BS=4, S=2048, H=1024), per batch b, per s-tile t (512):
  PE:  v_projT[ho] = sum_k Wv[k,ho]^T @ valuesT[k, t]   (f32r, N=512)
  ACT: featT = tanh(v_projT + (q_proj[b] + bv) per-partition bias)  -> f32r
  PE:  scores[1,512] += Wc[ho]^T @ featT
  DVE: scores += mask_bias;  ACT: p_t = exp(scores_t), accum_out -> tile sum
  PE:  broadcast p_t to 128 partitions (ones[1,128]^T @ p_t outer product)
  DVE: ctx_acc[:, k,t] = sum_j valuesT[k-slice] * p_t   (tensor_tensor_reduce)
Tail per batch: Z = sum(tile sums); w = p/Z (ACT scale); unnormalized ctxT and
Z are exported, the final ctx/Z division happens host-side during unsharding.
No max-subtraction is needed: |scores| <= ||Wc||_1 + eps ~ 16, exp is safe in f32,
which makes softmax+context tile-local (single pass over values, no re-read).
bc is dropped: softmax and context are invariant to a scalar score offset.
"""

import sys

for _p in ("/opt/trn_rl_repo",):
    if _p not in sys.path:
        sys.path.insert(0, _p)

import numpy as np

import concourse.bass as bass  # noqa: F401
import concourse.mybir as mybir
import concourse.tile as tile
from concourse import bacc
from concourse.bass_utils import run_bass_kernel_spmd

B, S, H = 32, 2048, 1024
NCORES = 8
BS = B // NCORES  # batches per core
KC = H // 128     # 8 contraction chunks
HO = H // 128     # 8 output-h chunks
ST = S // 512     # 4 s-tiles
F32 = mybir.dt.float32
F32R = mybir.dt.float32r
BF16 = mybir.dt.bfloat16

TRACE = False
LAST_RESULT = None

_cache = {}


def _build():
    AF = mybir.ActivationFunctionType
    ALU = mybir.AluOpType
    AX = mybir.AxisListType

    nc = bacc.Bacc("TRN2", target_bir_lowering=False, debug=False,
                   num_devices=NCORES)

    valT = nc.declare_dram_parameter("valT", [BS, H, S], BF16, isOutput=False).ap()
    qt = nc.declare_dram_parameter("qt", [128, KC * BS], BF16, isOutput=False).ap()
    wv = nc.declare_dram_parameter("wv", [128, KC * H], BF16, isOutput=False).ap()
    wq = nc.declare_dram_parameter("wq", [128, KC * H], BF16, isOutput=False).ap()
    wc = nc.declare_dram_parameter("wc", [128, KC], BF16, isOutput=False).ap()
    bqv = nc.declare_dram_parameter("bqv", [128, KC], F32, isOutput=False).ap()
    mkb = nc.declare_dram_parameter("mkb", [BS, S], F32, isOutput=False).ap()
    octx = nc.declare_dram_parameter("octx", [BS, 128, KC], F32, isOutput=True).ap()
    owgt = nc.declare_dram_parameter("owgt", [BS, S], F32, isOutput=True).ap()
    oz = nc.declare_dram_parameter("oz", [BS, 1], F32, isOutput=True).ap()

    with tile.TileContext(nc) as tc:
        import contextlib

        with contextlib.ExitStack() as es:
            const = es.enter_context(tc.tile_pool(name="const", bufs=1))
            psv = es.enter_context(tc.tile_pool(name="psv", bufs=6, space="PSUM"))
            pss = es.enter_context(tc.tile_pool(name="pss", bufs=2, space="PSUM"))

            # scalar queue: tiny consts first, then wv chunks, then vt stream.
            wc_sb = const.tile([128, KC], BF16, name="wc_sb")
            nc.scalar.dma_start(wc_sb[:], wc[:])
            bqv_sb = const.tile([128, KC], F32, name="bqv_sb")
            nc.scalar.dma_start(bqv_sb[:], bqv[:])
            qt_sb = const.tile([128, KC * BS], BF16, name="qt_sb")
            nc.scalar.dma_start(qt_sb[:], qt[:])
            qp_sb = const.tile([128, HO * BS], F32, name="qp_sb")
            valt_pool = es.enter_context(tc.tile_pool(name="valt", bufs=3))
            vt00 = valt_pool.tile([128, KC * 512], BF16, name="vt", tag="vt")
            nc.scalar.dma_start(
                vt00.rearrange("p (k j) -> p k j", k=KC),
                valT[0:1, :, 0:512].rearrange("o (k p) j -> (o p) k j", p=128),
            )
            wv_sb = const.tile([128, KC * H], BF16, name="wv_sb")
            for ho in range(HO):
                nc.scalar.dma_start(wv_sb[:, ho * H:(ho + 1) * H],
                                    wv[:, ho * H:(ho + 1) * H])

            # ---- q_projT: q_proj = query @ Wq, transpose via DRAM round-trip ----
            dramp = es.enter_context(tc.tile_pool(name="dramp", bufs=1, space="DRAM"))
            qpd = dramp.tile([BS, H], F32, name="qpd")
            with tc.tile_pool(name="wqp", bufs=1) as wqp:
                wq_sb = wqp.tile([128, KC * H], BF16, name="wq_sb")
                nc.sync.dma_start(wq_sb[:], wq[:])
                qpn = wqp.tile([BS, H], F32, name="qpn")
                for n in range(2):
                    ps_q = psv.tile([BS, 512], F32, name="ps_q", tag="psv")
                    for k in range(KC):
                        nc.tensor.matmul(
                            ps_q[:],
                            qt_sb[:, k * BS:(k + 1) * BS],
                            wq_sb[:, k * H + n * 512: k * H + n * 512 + 512],
                            start=(k == 0), stop=(k == KC - 1),
                        )
                    nc.vector.tensor_copy(qpn[:, n * 512:(n + 1) * 512], ps_q[:])
                nc.sync.dma_start(qpd[:], qpn[:])
            qpr = const.tile([128, HO * BS], F32, name="qpr")
            for c in range(HO):
                nc.sync.dma_start(
                    qpr[:, c * BS:(c + 1) * BS],
                    qpd[:, c * 128:(c + 1) * 128].rearrange("b p -> p b"),
                )
            for c in range(HO):
                nc.vector.tensor_scalar_add(
                    qp_sb[:, c * BS:(c + 1) * BS],
                    qpr[:, c * BS:(c + 1) * BS],
                    bqv_sb[:, c:c + 1],
                )

            ft_pool = es.enter_context(tc.tile_pool(name="ftp", bufs=4))
            sm_pool = es.enter_context(tc.tile_pool(name="sm", bufs=2))
            cx_pool = es.enter_context(tc.tile_pool(name="cx", bufs=2))

            for b in range(BS):
                mb_sb = sm_pool.tile([1, S], F32, name="mb_sb", tag="mb")
                nc.sync.dma_start(mb_sb[:], mkb[b:b + 1, :])
                p_sb = sm_pool.tile([1, S], F32, name="p_sb", tag="p")
                sums = sm_pool.tile([1, ST], F32, name="sums", tag="sums")
                ctr = cx_pool.tile([128, KC], F32, name="ctr", tag="ctr")

                for t in range(ST):
                    if b == 0 and t == 0:
                        vt = vt00
                    else:
                        vt = valt_pool.tile([128, KC * 512], BF16, name="vt",
                                            tag="vt")
                        nc.scalar.dma_start(
                            vt.rearrange("p (k j) -> p k j", k=KC),
                            valT[b:b + 1, :, t * 512:(t + 1) * 512]
                            .rearrange("o (k p) j -> (o p) k j", p=128),
                        )
                    ps_s = pss.tile([1, 512], F32, name="ps_s", tag="pss")
                    fts = []
                    for ho in range(HO):
                        ps_v = psv.tile([128, 512], F32, name="ps_v", tag="psv")
                        for k in range(KC):
                            nc.tensor.matmul(
                                ps_v[:],
                                wv_sb[:, ho * H + k * 128: ho * H + k * 128 + 128],
                                vt[:, k * 512:(k + 1) * 512],
                                start=(k == 0), stop=(k == KC - 1),
                            )
                        ft = ft_pool.tile([128, 512], BF16, name="ft", tag="ft")
                        nc.scalar.activation(
                            ft[:], ps_v[:], AF.Tanh,
                            bias=qp_sb[:, ho * BS + b: ho * BS + b + 1],
                        )
                        fts.append(ft)
                        # scores MM for ho-1: its tanh finished during this group
                        if ho >= 1:
                            nc.tensor.matmul(
                                ps_s[:], wc_sb[:, ho - 1:ho], fts[ho - 1][:],
                                start=(ho == 1), stop=False,
                            )
                    nc.tensor.matmul(
                        ps_s[:], wc_sb[:, HO - 1:HO], fts[HO - 1][:],
                        start=False, stop=True,
                    )
                    # scores -> masked -> exp (with per-tile sum) -> broadcast
                    sc_t = sm_pool.tile([1, 512], F32, name="sc_t", tag="sct")
                    nc.vector.tensor_add(
                        sc_t[:], ps_s[:], mb_sb[:, t * 512:(t + 1) * 512])
                    nc.scalar.activation(
                        p_sb[:, t * 512:(t + 1) * 512], sc_t[:], AF.Exp,
                        accum_out=sums[:, t:t + 1],
                    )
                    prep = cx_pool.tile([128, 512], F32, name="prep", tag="prep")
                    nc.gpsimd.partition_broadcast(
                        prep[:], p_sb[:, t * 512:(t + 1) * 512])
                    tmp = cx_pool.tile([128, KC * 512], F32, name="tmp", tag="tmp")
                    ctat = cx_pool.tile([128, KC], F32, name="ctat", tag="ctat")
                    kk = KC // 2
                    for h2 in range(2):
                        sl = slice(h2 * kk * 512, (h2 + 1) * kk * 512)
                        nc.vector.tensor_mul(
                            tmp[:, sl].rearrange("p (k j) -> p k j", k=kk),
                            vt[:, sl].rearrange("p (k j) -> p k j", k=kk),
                            prep.rearrange("p (o j) -> p o j", o=1)
                            .broadcast_to([128, kk, 512]),
                        )
                        nc.vector.tensor_reduce(
                            ctat[:, h2 * kk:(h2 + 1) * kk],
                            tmp[:, sl].rearrange("p (k j) -> p k j", k=kk),
                            axis=AX.X, op=ALU.add,
                        )
                    if t == 0:
                        nc.vector.tensor_copy(ctr[:], ctat[:])
                    else:
                        nc.vector.tensor_add(ctr[:], ctr[:], ctat[:])

                # ---------- batch tail: normalize ----------
                tot = sm_pool.tile([1, 1], F32, name="tot", tag="tot")
                nc.vector.tensor_reduce(tot[:], sums[:], axis=AX.X, op=ALU.add)
                recip = sm_pool.tile([1, 1], F32, name="recip", tag="recip")
                nc.vector.reciprocal(recip[:], tot[:])
                w_sb = sm_pool.tile([1, S], F32, name="w_sb", tag="w")
                nc.scalar.mul(w_sb[:], p_sb[:], recip[:, 0:1])
                nc.sync.dma_start(owgt[b:b + 1, :], w_sb[:])
                nc.sync.dma_start(oz[b:b + 1, :], tot[:])

                nc.sync.dma_start(
                    octx[b:b + 1].rearrange("o p k -> (o p) k"),
                    ctr[:],
                )

    nc.compile()
    return nc


def _prep_host(query, values, mask, Wq, bq, Wv, bv, Wc, bc):
    f32 = np.float32
    query = np.asarray(query, f32)
    values = np.asarray(values, f32)
    mask = np.asarray(mask)
    Wq = np.asarray(Wq, f32)
    Wv = np.asarray(Wv, f32)
    Wc = np.asarray(Wc, f32)
    bq = np.asarray(bq, f32)
    bv = np.asarray(bv, f32)

    import ml_dtypes
    bf16 = ml_dtypes.bfloat16
    # wv: ho-major chunked layout [p, ho*H + k*128 + c] = Wv[k*128+p, ho*128+c]
    wv_l = np.ascontiguousarray(
        Wv.reshape(KC, 128, HO, 128).transpose(1, 2, 0, 3)
        .reshape(128, KC * H).astype(bf16))
    wq_l = np.ascontiguousarray(
        Wq.reshape(KC, 128, H).transpose(1, 0, 2).reshape(128, KC * H)
        .astype(bf16))
    wc_l = np.ascontiguousarray(Wc.reshape(KC, 128).T.astype(bf16))
    bqv_l = np.ascontiguousarray((bq + bv).reshape(KC, 128).T)  # [128, KC]

    valuesT = np.ascontiguousarray(values.transpose(0, 2, 1).astype(bf16))
    mask_bias = np.where(mask, 0.0, -1e38).astype(f32)          # [B, S]

    in_maps = []
    for c in range(NCORES):
        lo, hi = c * BS, (c + 1) * BS
        qs = query[lo:hi]
        qt_l = np.ascontiguousarray(
            qs.reshape(BS, KC, 128).transpose(2, 1, 0).reshape(128, KC * BS)
            .astype(bf16))
        in_maps.append({
            "valT": np.ascontiguousarray(valuesT[lo:hi]),
            "qt": qt_l,
            "wv": wv_l,
            "wq": wq_l,
            "wc": wc_l,
            "bqv": bqv_l,
            "mkb": np.ascontiguousarray(mask_bias[lo:hi]),
        })
    return in_maps


def kernel(query, values, mask, Wq, bq, Wv, bv, Wc, bc):
    global LAST_RESULT
    if "nc" not in _cache:
        _cache["nc"] = _build()
    nc = _cache["nc"]

    in_maps = _prep_host(query, values, mask, Wq, bq, Wv, bv, Wc, bc)
    res = run_bass_kernel_spmd(nc, in_maps, core_ids=list(range(NCORES)),
                               trace=TRACE)
    LAST_RESULT = res
    outs = res.results
    context = np.concatenate([outs[c]["octx"] for c in range(NCORES)], axis=0)
    context = context.transpose(0, 2, 1).reshape(B, H)  # [b,p,k] -> h = k*128+p
    weights = np.concatenate([outs[c]["owgt"] for c in range(NCORES)], axis=0)
    z = np.concatenate([outs[c]["oz"] for c in range(NCORES)], axis=0)
    context = context / z
    return context.astype(np.float32), weights.astype(np.float32)
